# revision 1
# baseline (speedup 1.0000x reference)
"""Trainium2 Bass kernel for DiffusionPropers (gnn_message_passing), v3.

Baseline table/SWDGE architecture with:
  - DVE bf16 Z-sum (1 PE transpose/block instead of 4 transposing matmuls)
  - x3-stationary delta matmuls (drops dps/dsb/dtp chain)
  - broadcast-view coord copies in phase 0 (1 op instead of 4)
  - replicated-xyz cross products and broadcast muls in geometry
  - s0/s3 read the delta PSUM directly
"""
import numpy as np
import ml_dtypes

# ---------------- compile-time constants (hardcoded problem shape) --------
N_ATOMS = 25000
NA = 25088              # padded atoms (196 * 128)
P_TOT = 100000
T_STEPS = 4
D = 128
DIN = 516
N_CORES = 8
PPC = 12500             # real props per core
PPCT = 12544            # padded props per core (98 tiles of 128)
NTILES = PPCT // 128    # 98
CH = 896                # props per gather/scatter call (SWDGE ring limit)
NCHUNK = PPCT // CH     # 14
CBLK = CH // 128        # 7
SLAB = 256              # bf16 elems per table slab (512B)
DUMP = NA               # scatter dump row
A_ROWS = NA + 8         # accumulator rows (incl. dump)
A_COLS = 64             # 256B stride for scatter
LEAKY = 0.001

_BF16 = ml_dtypes.bfloat16

_compiled = None


# ------------------------- host-side helpers ------------------------------

def _wrap_idxs(idx: np.ndarray) -> np.ndarray:
    """[n] int -> [128, n/16] int16, wrapped in 16 partitions, replicated x8."""
    n = idx.shape[0]
    assert n % 16 == 0
    w = idx.reshape(-1, 16).T.astype(np.int16)
    return np.tile(w, (8, 1))


def _order_props(props: np.ndarray, n_real: int, seed: int = 0) -> np.ndarray:
    """Order PPCT props so that within every aligned CH-chunk the p0 targets
    are distinct and the p3 targets are distinct (scatter-add race freedom)."""
    n = props.shape[0]
    rng = np.random.default_rng(seed)
    for attempt in range(50):
        perm = rng.permutation(n_real)
        buckets: list[list[int]] = [[] for _ in range(NCHUNK)]
        used0: list[set] = [set() for _ in range(NCHUNK)]
        used3: list[set] = [set() for _ in range(NCHUNK)]
        fail = []
        start = 0
        for j in perm:
            a0 = int(props[j, 0])
            a3 = int(props[j, 3])
            for d in range(NCHUNK):
                b = (start + d) % NCHUNK
                if (len(buckets[b]) < CH and a0 not in used0[b]
                        and a3 not in used3[b]):
                    buckets[b].append(int(j))
                    used0[b].add(a0)
                    used3[b].add(a3)
                    break
            else:
                fail.append(int(j))
            start = (start + 1) % NCHUNK
        if fail:
            continue
        pads = list(range(n_real, n))
        for b in range(NCHUNK):
            while len(buckets[b]) < CH:
                buckets[b].append(pads.pop())
        assert not pads
        order = [j for b in buckets for j in b]
        return np.array(order, dtype=np.int64)
    raise RuntimeError("prop ordering failed")


# ------------------------- device kernel build ----------------------------

def _build():
    import concourse.bass as bass
    import concourse.bacc as bacc
    import concourse.mybir as mybir
    import concourse.tile as tile
    from concourse.masks import make_identity
    from concourse.library_config import mlp as mlp_lib

    F32 = mybir.dt.float32
    BF16 = mybir.dt.bfloat16
    I16 = mybir.dt.int16
    AF = mybir.ActivationFunctionType

    nc = bacc.Bacc("TRN2", target_bir_lowering=False, debug=False,
                   num_devices=N_CORES)

    # ---- I/O ----
    encT = nc.dram_tensor("encT", [D, NA], BF16, kind="ExternalInput")
    coordsb = nc.dram_tensor("coordsb", [NA, 24], BF16, kind="ExternalInput")
    w0all = nc.dram_tensor("w0all", [D, 512], BF16, kind="ExternalInput")
    wmisc = nc.dram_tensor("wmisc", [16, 512], BF16, kind="ExternalInput")
    w1 = nc.dram_tensor("w1", [D, D], BF16, kind="ExternalInput")
    w2 = nc.dram_tensor("w2", [D, D], BF16, kind="ExternalInput")
    w3 = nc.dram_tensor("w3", [D, 2], BF16, kind="ExternalInput")
    bias12 = nc.dram_tensor("bias12", [D, 2], F32, kind="ExternalInput")
    b3h = nc.dram_tensor("b3h", [D, 2], F32, kind="ExternalInput")
    gidx = nc.dram_tensor("gidx", [128, 4 * (PPCT // 16)], I16,
                          kind="ExternalInput")
    sidx = nc.dram_tensor("sidx", [128, 2 * (PPCT // 16)], I16,
                          kind="ExternalInput")
    A0 = nc.dram_tensor("A0", [A_ROWS, A_COLS], F32, kind="ExternalOutput")
    A3 = nc.dram_tensor("A3", [A_ROWS, A_COLS], F32, kind="ExternalOutput")
    Tt = nc.dram_tensor("Tt", [4, NA, SLAB], BF16)   # internal tables

    GI = PPCT // 16     # 784

    with tile.TileContext(nc) as tc:
        with (
            tc.tile_pool(name="const", bufs=1) as cpool,
        ):
            nc.gpsimd.load_library(mlp_lib)

            # ---- constants ----
            ibf = cpool.tile([128, 128], BF16)
            make_identity(nc, ibf[:])
            if32 = cpool.tile([128, 128], F32)
            make_identity(nc, if32[:])
            zero_b = cpool.tile([128, 1], F32)
            nc.vector.memset(zero_b[:], 0.0)
            eps_b = cpool.tile([128, 1], F32)
            nc.vector.memset(eps_b[:], 1e-12)
            negh = cpool.tile([128, 1], F32)
            nc.vector.memset(negh[:], -0.5)
            posh = cpool.tile([128, 1], F32)
            nc.vector.memset(posh[:], 0.5)

            w0t = cpool.tile([D, 512], BF16)
            nc.sync.dma_start(out=w0t[:], in_=w0all[:])
            wmt = cpool.tile([16, 512], BF16)
            nc.sync.dma_start(out=wmt[:], in_=wmisc[:])
            w1t = cpool.tile([D, D], BF16)
            nc.sync.dma_start(out=w1t[:], in_=w1[:])
            w2t = cpool.tile([D, D], BF16)
            nc.sync.dma_start(out=w2t[:], in_=w2[:])
            w3t = cpool.tile([D, 2], BF16)
            nc.sync.dma_start(out=w3t[:], in_=w3[:])
            b12t = cpool.tile([D, 2], F32)
            nc.sync.dma_start(out=b12t[:], in_=bias12[:])
            b3t = cpool.tile([D, 2], F32)
            nc.sync.dma_start(out=b3t[:], in_=b3h[:])
            gixt = cpool.tile([128, 4 * GI], I16)
            nc.sync.dma_start(out=gixt[:], in_=gidx[:])
            sixt = cpool.tile([128, 2 * GI], I16)
            nc.sync.dma_start(out=sixt[:], in_=sidx[:])


            # ================= Phase 0: build table =================
            SC = 2048
            with (
                tc.tile_pool(name="p0", bufs=3) as p0pool,
                tc.tile_pool(name="p0ps", bufs=8, space="PSUM") as p0ps,
            ):
                cob = p0pool.tile([128, NA // 128, 24], BF16, tag="cob")
                nc.sync.dma_start(
                    out=cob[:],
                    in_=coordsb[:].rearrange("(b p) c -> p b c", p=128))
                nsc = NA // SC
                rem = NA - nsc * SC
                spans = [(i * SC, SC) for i in range(nsc)]
                if rem:
                    spans.append((nsc * SC, rem))
                for base, ln in spans:
                    et = p0pool.tile([128, SC], BF16, tag="et")
                    nc.sync.dma_start(out=et[:, :ln], in_=encT[:, base:base + ln])
                    asm = p0pool.tile([128, SC // 128, 4, 152], BF16, tag="asm")
                    for s in range(ln // 128):
                        blk = base // 128 + s
                        ps = p0ps.tile([128, 512], F32, tag="yps")
                        nc.tensor.matmul(ps[:], lhsT=et[:, s * 128:(s + 1) * 128],
                                         rhs=w0t[:], start=True, stop=True)
                        psv = ps[:].rearrange("p (a b) -> p a b", a=4)
                        if s % 2 == 0:
                            nc.scalar.activation(asm[:, s, :, 0:128], psv, AF.Copy)
                        else:
                            nc.vector.tensor_copy(asm[:, s, :, 0:128], psv)
                        for k in range(4):
                            nc.vector.tensor_copy(asm[:, s, k, 128:152],
                                                  cob[:, blk, :])
                    for k in range(4):
                        eng = nc.sync if k % 2 == 0 else nc.gpsimd
                        eng.dma_start(
                            out=Tt[k, base:base + ln, 0:152].rearrange(
                                "(s p) e -> p s e", p=128),
                            in_=asm[:, :ln // 128, k, :])

            # ================= Phase 1: main loop =================
            with (
                tc.tile_pool(name="mn", bufs=4) as mpool,
                tc.tile_pool(name="geo", bufs=2) as gpool,
                tc.tile_pool(name="cto", bufs=3) as ctpool,
                tc.tile_pool(name="ps1", bufs=2, space="PSUM") as ps1,
                tc.tile_pool(name="ps2", bufs=1, space="PSUM") as ps2,
            ):
                Gof = {}
                ctof = {}

                def do_gather(c):
                    G = []
                    for k in range(4):
                        g = mpool.tile([128, CBLK, SLAB], BF16, tag=f"g{k}")
                        nc.gpsimd.dma_gather(
                            g[:], Tt[k],
                            gixt[:, k * GI + c * (CH // 16):
                                 k * GI + (c + 1) * (CH // 16)],
                            CH, CH, SLAB)
                        G.append(g)
                    Gof[c] = G

                def do_compute(c):
                    G = Gof[c]
                    cco = [G[k][:, :, 128:152].bitcast(F32) for k in range(4)]

                    # ---- Z sum on DVE (bf16) ----
                    Zp = mpool.tile([128, CBLK, 128], BF16, tag="Zp")
                    nc.vector.tensor_add(Zp[:], G[0][:, :, 0:128],
                                         G[1][:, :, 0:128])
                    nc.vector.tensor_add(Zp[:], Zp[:], G[2][:, :, 0:128])
                    nc.vector.tensor_add(Zp[:], Zp[:], G[3][:, :, 0:128])

                    # ---- geometry (baseline flat-view style) ----
                    u1 = gpool.tile([128, CBLK, 12], F32, tag="u1")
                    u2 = gpool.tile([128, CBLK, 12], F32, tag="u2")
                    u3 = gpool.tile([128, CBLK, 12], F32, tag="u3")
                    dr = gpool.tile([128, CBLK, 12], F32, tag="dr")
                    nc.vector.tensor_sub(u1[:], cco[1], cco[0])
                    nc.vector.tensor_sub(u2[:], cco[2], cco[1])
                    nc.vector.tensor_sub(u3[:], cco[3], cco[2])
                    nc.vector.tensor_sub(dr[:], cco[0], cco[3])

                    def cross(out, a, b):
                        tmp = gpool.tile([128, CBLK, 4], F32, tag="ctmp")
                        for x in range(3):
                            y, z = (x + 1) % 3, (x + 2) % 3
                            nc.vector.tensor_mul(tmp[:], a[:, :, y::3],
                                                 b[:, :, z::3])
                            nc.vector.tensor_mul(out[:, :, x::3],
                                                 a[:, :, z::3], b[:, :, y::3])
                            nc.vector.tensor_sub(out[:, :, x::3], tmp[:],
                                                 out[:, :, x::3])

                    cr12 = gpool.tile([128, CBLK, 12], F32, tag="cr12")
                    cr23 = gpool.tile([128, CBLK, 12], F32, tag="cr23")
                    cross(cr12, u1, u2)
                    cross(cr23, u2, u3)

                    def dot3(out, a, b, tmp):
                        nc.vector.tensor_mul(tmp[:], a[:], b[:])
                        nc.vector.tensor_add(out[:], tmp[:, :, 0::3],
                                             tmp[:, :, 1::3])
                        nc.vector.tensor_add(out[:], out[:], tmp[:, :, 2::3])

                    tmp12 = gpool.tile([128, CBLK, 12], F32, tag="tmp12")
                    n2 = gpool.tile([128, CBLK, 4], F32, tag="n2")
                    dot3(n2, u2, u2, tmp12)
                    nc.scalar.activation(n2[:], n2[:], AF.Sqrt, bias=zero_b[:])
                    sn = gpool.tile([128, CBLK, 4], F32, tag="sn")
                    dot3(sn, u1, cr23, tmp12)
                    nc.vector.tensor_mul(sn[:], sn[:], n2[:])
                    cn = gpool.tile([128, CBLK, 4], F32, tag="cn")
                    dot3(cn, cr12, cr23, tmp12)
                    hy = gpool.tile([128, CBLK, 4], F32, tag="hy")
                    t2 = gpool.tile([128, CBLK, 4], F32, tag="t2")
                    nc.vector.tensor_mul(hy[:], sn[:], sn[:])
                    nc.vector.tensor_mul(t2[:], cn[:], cn[:])
                    nc.vector.tensor_add(hy[:], hy[:], t2[:])
                    nc.scalar.activation(hy[:], hy[:], AF.Sqrt, bias=eps_b[:])
                    rh = gpool.tile([128, CBLK, 4], F32, tag="rh")
                    nc.vector.reciprocal(rh[:], hy[:])
                    dl = gpool.tile([128, CBLK, 4], F32, tag="dl")
                    dot3(dl, dr, dr, tmp12)
                    nc.scalar.activation(dl[:], dl[:], AF.Sqrt, bias=eps_b[:])
                    rdl = gpool.tile([128, CBLK, 4], F32, tag="rdl")
                    nc.vector.reciprocal(rdl[:], dl[:])
                    dh = gpool.tile([128, CBLK, 12], F32, tag="dh")
                    for x in range(3):
                        nc.vector.tensor_mul(dh[:, :, x::3], dr[:, :, x::3],
                                             rdl[:])
                    geo = gpool.tile([128, CBLK, 16], F32, tag="geo")
                    nc.vector.memset(geo[:], 1.0)
                    sincos = gpool.tile([128, CBLK, 4], F32, tag="sc0")
                    nc.vector.tensor_mul(sincos[:], sn[:], rh[:])
                    nc.vector.tensor_copy(geo[:, :, 0::4], sincos[:])
                    nc.vector.tensor_mul(sincos[:], cn[:], rh[:])
                    nc.vector.tensor_copy(geo[:, :, 1::4], sincos[:])
                    nc.vector.tensor_copy(geo[:, :, 2::4], dl[:])

                    # ---- per-block MLP ----
                    c0t = ctpool.tile([128, CBLK, 12], F32, tag="c0t")
                    c3t = ctpool.tile([128, CBLK, 12], F32, tag="c3t")
                    dtc = ps2.tile([128, CBLK, 4, 2], F32, tag="dtc")
                    for b in range(CBLK):
                        # Z^T via 1 transpose matmul
                        zps = ps2.tile([128, 128], F32, tag="z")
                        nc.tensor.matmul(zps[:], lhsT=Zp[:, b, :], rhs=ibf[:],
                                         start=True, stop=True)
                        zbf = mpool.tile([128, 128], BF16, tag="zbf")
                        if b % 2 == 0:
                            nc.scalar.activation(zbf[:], zps[:], AF.Copy)
                        else:
                            nc.vector.tensor_copy(zbf[:], zps[:])
                        # geoT [16, 128]
                        gtp = ps2.tile([16, 128], F32, tag="gt")
                        nc.tensor.matmul(
                            gtp[:],
                            lhsT=geo[:, b, :],
                            rhs=if32[:], is_transpose=True,
                            start=True, stop=True)
                        gtb = mpool.tile([16, 128], BF16, tag="gtb")
                        nc.vector.tensor_copy(gtb[:], gtp[:])
                        # h1 = Z bcast + misc
                        h1 = ps1.tile([128, 512], F32, tag="hA")
                        for ti in range(4):
                            nc.tensor.matmul(
                                h1[:, ti * 128:(ti + 1) * 128],
                                lhsT=ibf[:], rhs=zbf[:],
                                start=True, stop=False)
                            nc.tensor.matmul(
                                h1[:, ti * 128:(ti + 1) * 128],
                                lhsT=wmt[:, ti * 128:(ti + 1) * 128],
                                rhs=gtb[:], start=False, stop=True)
                        x1 = mpool.tile([128, 512], BF16, tag="x1")
                        nc.scalar.activation(x1[:], h1[:], AF.Prelu,
                                             bias=zero_b[:], alpha=LEAKY)
                        h2 = ps1.tile([128, 512], F32, tag="hB")
                        nc.tensor.matmul(h2[:], lhsT=w1t[:], rhs=x1[:],
                                         start=True, stop=True)
                        x2 = mpool.tile([128, 512], BF16, tag="x2")
                        nc.scalar.activation(x2[:], h2[:], AF.Prelu,
                                             bias=b12t[:, 0:1], alpha=LEAKY)
                        h3 = ps1.tile([128, 512], F32, tag="hA")
                        nc.tensor.matmul(h3[:], lhsT=w2t[:], rhs=x2[:],
                                         start=True, stop=True)
                        x3 = mpool.tile([128, 512], BF16, tag="x3")
                        nc.scalar.activation(x3[:], h3[:], AF.Prelu,
                                             bias=b12t[:, 1:2], alpha=LEAKY)
                        # delta: x3-stationary [props, 2] per ti
                        for ti in range(4):
                            nc.tensor.matmul(
                                dtc[:, b, ti, :],
                                lhsT=x3[:, ti * 128:(ti + 1) * 128],
                                rhs=w3t[:], start=True, stop=True)

                    # corrections (whole chunk)
                    s0 = gpool.tile([128, CBLK, 4], F32, tag="s0")
                    s3 = gpool.tile([128, CBLK, 4], F32, tag="s3")
                    nc.vector.tensor_scalar(
                        s0[:], dtc[:, :, :, 0], scalar1=negh[:],
                        scalar2=b3t[:, 0:1],
                        op0=mybir.AluOpType.mult, op1=mybir.AluOpType.add)
                    nc.vector.tensor_scalar(
                        s3[:], dtc[:, :, :, 1], scalar1=posh[:],
                        scalar2=b3t[:, 1:2],
                        op0=mybir.AluOpType.mult, op1=mybir.AluOpType.add)
                    for x in range(3):
                        nc.vector.tensor_mul(c0t[:, :, x::3], dh[:, :, x::3],
                                             s0[:])
                        nc.vector.tensor_mul(c3t[:, :, x::3], dh[:, :, x::3],
                                             s3[:])
                    ctof[c] = (c0t, c3t)

                def do_scatter(c):
                    c0t, c3t = ctof.pop(c)
                    nc.gpsimd.dma_scatter_add(
                        A0[:, :12], c0t[:],
                        sixt[:, c * (CH // 16):(c + 1) * (CH // 16)],
                        CH, CH, 12, elem_step=A_COLS)
                    nc.gpsimd.dma_scatter_add(
                        A3[:, :12], c3t[:],
                        sixt[:, GI + c * (CH // 16):GI + (c + 1) * (CH // 16)],
                        CH, CH, 12, elem_step=A_COLS)
                    del Gof[c]

                do_gather(0)
                do_gather(1)
                for c in range(2, NCHUNK):
                    do_gather(c)
                    do_compute(c - 2)
                    if c >= 3:
                        do_scatter(c - 3)
                do_compute(NCHUNK - 2)
                do_scatter(NCHUNK - 3)
                do_compute(NCHUNK - 1)
                do_scatter(NCHUNK - 2)
                do_scatter(NCHUNK - 1)

    nc.compile()
    return nc


def _get_compiled():
    global _compiled
    if _compiled is None:
        _compiled = _build()
    return _compiled


# ------------------------------ entry point -------------------------------

def _prep_in_maps(coords, propers, encoded, t, answer, W0, b0, W1, b1, W2, b2,
                  W3, b3):
    coords = np.asarray(coords, dtype=np.float32)
    propers_np = np.asarray(propers)
    encoded = np.asarray(encoded, dtype=np.float32)
    t = np.asarray(t, dtype=np.float32)
    W0 = np.asarray(W0, dtype=np.float32)
    b0 = np.asarray(b0, dtype=np.float32)
    W1 = np.asarray(W1, dtype=np.float32)
    b1 = np.asarray(b1, dtype=np.float32)
    W2 = np.asarray(W2, dtype=np.float32)
    b2 = np.asarray(b2, dtype=np.float32)
    W3 = np.asarray(W3, dtype=np.float32)
    b3 = np.asarray(b3, dtype=np.float32)

    encT = np.zeros((D, NA), dtype=_BF16)
    encT[:, :N_ATOMS] = encoded.T.astype(_BF16)
    cflat = np.zeros((NA, 12), dtype=np.float32)
    cflat[:N_ATOMS] = coords.reshape(N_ATOMS, 12)
    coordsb = cflat.view(np.uint16).view(_BF16)

    w0all = np.concatenate([W0[128 * k:128 * (k + 1)] for k in range(4)],
                           axis=1).astype(_BF16)
    wmisc = np.zeros((16, 512), dtype=np.float32)
    for ti in range(T_STEPS):
        wmisc[4 * ti + 0, ti * 128:(ti + 1) * 128] = W0[513]
        wmisc[4 * ti + 1, ti * 128:(ti + 1) * 128] = W0[514]
        wmisc[4 * ti + 2, ti * 128:(ti + 1) * 128] = W0[515]
        wmisc[4 * ti + 3, ti * 128:(ti + 1) * 128] = b0 + t[ti] * W0[512]
    wmisc = wmisc.astype(_BF16)
    bias12 = np.stack([b1, b2], axis=1).astype(np.float32)
    b3h = np.zeros((D, 2), dtype=np.float32)
    b3h[:, 0] = -0.5 * b3[0]
    b3h[:, 1] = 0.5 * b3[1]

    shared = {
        "encT": encT,
        "coordsb": np.ascontiguousarray(coordsb),
        "w0all": w0all,
        "wmisc": wmisc,
        "w1": W1.astype(_BF16),
        "w2": W2.astype(_BF16),
        "w3": W3.astype(_BF16),
        "bias12": bias12,
        "b3h": b3h,
    }

    props32 = propers_np.astype(np.int32)
    in_maps = []
    for cidx in range(N_CORES):
        shard = np.zeros((PPCT, 4), dtype=np.int32)
        shard[:PPC] = props32[cidx * PPC:(cidx + 1) * PPC]
        order = _order_props(shard, PPC, seed=cidx)
        po = shard[order]
        is_pad = order >= PPC
        gi = np.concatenate([_wrap_idxs(po[:, k]) for k in range(4)], axis=1)
        tgt0 = np.where(is_pad, DUMP, po[:, 0]).astype(np.int32)
        tgt3 = np.where(is_pad, DUMP, po[:, 3]).astype(np.int32)
        si = np.concatenate([_wrap_idxs(tgt0), _wrap_idxs(tgt3)], axis=1)
        in_maps.append({**shared, "gidx": gi, "sidx": si})
    return in_maps


def kernel(coords, propers, encoded, t, answer, W0, b0, W1, b1, W2, b2, W3, b3,
           _trace=False):
    from concourse.bass_utils import run_bass_kernel_spmd

    answer = np.asarray(answer, dtype=np.float32)
    in_maps = _prep_in_maps(coords, propers, encoded, t, answer, W0, b0, W1,
                            b1, W2, b2, W3, b3)
    nc = _get_compiled()
    res = run_bass_kernel_spmd(nc, in_maps, core_ids=list(range(N_CORES)),
                               trace=_trace)
    if _trace:
        kernel.last_exec_ns = res.exec_time_ns
        kernel.last_results = res

    acc = np.zeros((N_ATOMS, 12), dtype=np.float32)
    for cidx in range(N_CORES):
        acc += res.results[cidx]["A0"][:N_ATOMS, :12]
        acc += res.results[cidx]["A3"][:N_ATOMS, :12]
    out = answer + acc.reshape(N_ATOMS, T_STEPS, 3)
    return out.astype(np.float32)


kernel.last_exec_ns = None
kernel.last_results = None

